# revision 1
# baseline (speedup 1.0000x reference)
"""Dehazing kernel for AWS Trainium2 (Bass/Tile), 8-core data-parallel.

Problem: img [32,3,512,512] f32, w [32] f32 ->
  dc  = 15x15 box-mean of per-pixel channel-min (zero-padded, /225)
  A_c = mean of img_c at the top-5% dc positions (k=13107 per image)
  t   = max(1 - w*dc, 0.1); out = clip((img-A)/(t+0.001) + A, 0, 1)

Sharding: pure data-parallel, batch 32 -> 8 NeuronCores x 4 images.
Per core, per image:
  - channel-min on DVE (2 tensor_tensor min ops)
  - horizontal 15-tap box sum: DVE prefix-scan + shifted subtract
    (+ small edge patches; zero-pad semantics match avg_pool2d
    count_include_pad)
  - vertical 15-tap box sum: PE banded-matrix matmuls (0/1 band
    matrices passed in as a constant input tensor); PSUM->SBUF copy
    applies the 1/225 scale on the Scalar engine
  - top-5% threshold: two-level per-partition "stripe grid" seed
    (one fused count pass per level) then 12 exact bisection rounds;
    each count is split DVE (lower half, is_ge+accum) / ACT (upper
    half, Sign+accum); cross-partition count reduction via a
    ones-matrix matmul on the (otherwise idle) tensor engine, which
    also broadcasts the total to all partitions
  - masked channel sums: fused scalar_tensor_tensor with accum_out
  - A = S/count (count==k except for sub-ULP ties, where the full tie
    set is averaged); dehaze: fused DVE ops, Relu(+A) on ACT,
    min-clamp on DVE, store in-place over the img tiles
"""
import os
import numpy as np

import concourse.bacc as bacc
import concourse.tile as tile
import concourse.mybir as mybir
from concourse.bass_utils import run_bass_kernel_spmd

F32 = mybir.dt.float32
I32 = mybir.dt.int32
U32 = mybir.dt.uint32
ALU = mybir.AluOpType
ACTF = mybir.ActivationFunctionType

P = 128
H = W = 512
G = H // P              # 4 row-groups
NPC = 4                 # images per core
K = 13107               # int(512*512*0.05)
KF = float(K)
NDVE_CNT = 896             # DVE count slice (per partition)
NACT_CNT = 2048 - NDVE_CNT  # ACT count slice (per partition)
NACT_TOT = float(NACT_CNT * P)
NTOT = float(H * W)
FULL_ROUNDS = 12

# grid-seed constants
D1 = 1.0 / 128.0            # level-1 grid step over [0,1)
L1_OFF = -3.0               # lo1 = (jcnt1 + L1_OFF) * D1
W1 = 5.0 * D1               # level-1 bracket width
D2 = W1 / 128.0             # level-2 grid step
L2_OFF = -4.0               # lo2 = lo1 + (jcnt2 + L2_OFF) * D2
W2 = 7.0 * D2               # bracket width entering full rounds
THR_DVE = KF / 128.0        # per-stripe count threshold (is_ge counts)
THR_ACT = 2.0 * KF / 128.0 - 2048.0  # same in sign-sum units


def make_consts() -> np.ndarray:
    k = np.arange(P)[:, None]
    m = np.arange(P)[None, :]
    bdiag = (np.abs(k - m) <= 7).astype(np.float32)
    bup = ((k - m) >= 121).astype(np.float32)
    bdn = ((m - k) >= 121).astype(np.float32)
    ones = np.ones((P, P), dtype=np.float32)
    return np.concatenate([bdiag, bup, bdn, ones], axis=1)  # [128, 512]


def build(nc):
    img_in = nc.dram_tensor("img", [NPC, 3, H, W], F32, kind="ExternalInput").ap()
    w_in = nc.dram_tensor("w", [NPC], F32, kind="ExternalInput").ap()
    consts_in = nc.dram_tensor("consts", [P, 4 * P], F32, kind="ExternalInput").ap()
    out_d = nc.dram_tensor("out", [NPC, 3, H, W], F32, kind="ExternalOutput").ap()

    with tile.TileContext(nc) as tc:
        with (
            tc.tile_pool(name="const", bufs=1) as const_pool,
            tc.tile_pool(name="img", bufs=4) as img_pool,
            tc.tile_pool(name="dcp", bufs=4) as dc_pool,
            tc.tile_pool(name="wk1p", bufs=2) as wk1p,
            tc.tile_pool(name="work", bufs=1) as work,
            tc.tile_pool(name="pbp", bufs=1) as pbp,
            tc.tile_pool(name="scnt", bufs=2) as scnt,
            tc.tile_pool(name="scnt2", bufs=2) as scnt2,
            tc.tile_pool(name="small", bufs=4) as small,
            tc.tile_pool(name="vband", bufs=2, space="PSUM") as vband,
            tc.tile_pool(name="cntps", bufs=2, space="PSUM") as cnt_ps,
            tc.tile_pool(name="miscps", bufs=1, space="PSUM") as misc_ps,
        ):
            consts = const_pool.tile([P, 4 * P], F32)
            nc.sync.dma_start(consts[:], consts_in[:])
            bdiag = consts[:, 0:P]
            bup = consts[:, P:2 * P]
            bdn = consts[:, 2 * P:3 * P]
            ones = consts[:, 3 * P:4 * P]

            # full-round combined compare: cnt_dve + 0.5*sum_act >= K - NHALF/2
            kvec_full = const_pool.tile([P, 2], F32)
            nc.vector.memset(kvec_full[:], KF - NACT_TOT / 2.0)

            # iota grid for the seed: g1[p] = p * D1 (and negated for ACT bias)
            grid_i = const_pool.tile([P, 1], I32)
            nc.gpsimd.iota(grid_i[:], pattern=[[0, 1]], base=0,
                           channel_multiplier=1)
            gridf = const_pool.tile([P, 1], F32)
            nc.vector.tensor_copy(gridf[:], grid_i[:])
            g1 = const_pool.tile([P, 1], F32)
            nc.vector.tensor_scalar(out=g1[:], in0=gridf[:], scalar1=D1,
                                    scalar2=None, op0=ALU.mult)
            ng1 = const_pool.tile([P, 1], F32)
            nc.vector.tensor_scalar(out=ng1[:], in0=g1[:], scalar1=-1.0,
                                    scalar2=None, op0=ALU.mult)

            w_sb = const_pool.tile([1, NPC], F32)
            nc.sync.dma_start(w_sb[:], w_in.rearrange("(p a) -> p a", p=1))
            w4_ps = misc_ps.tile([P, NPC], F32, tag="w4")
            nc.tensor.matmul(w4_ps[:], lhsT=ones[0:1, :], rhs=w_sb[:],
                             start=True, stop=True)
            negw4 = const_pool.tile([P, NPC], F32)
            nc.vector.tensor_scalar(out=negw4[:], in0=w4_ps[:], scalar1=-1.0,
                                    scalar2=None, op0=ALU.mult)

            def phase1(i):
                """load + channel-min + box filter -> (img tiles, dc tile)"""
                imgt = []
                for c in range(3):
                    t = img_pool.tile([P, G, W], F32, tag=f"img{c}")
                    nc.sync.dma_start(
                        t[:], img_in[i, c].rearrange("(g p) x -> p g x", p=P))
                    imgt.append(t)

                mn = wk1p.tile([P, G, W], F32, tag="wk1")
                nc.vector.tensor_tensor(out=mn[:], in0=imgt[0][:],
                                        in1=imgt[1][:], op=ALU.min)
                nc.vector.tensor_tensor(out=mn[:], in0=mn[:], in1=imgt[2][:],
                                        op=ALU.min)

                Pb = pbp.tile([P, 2056], F32, tag="pb")
                nc.vector.memset(Pb[:, 0:1], 0.0)
                mn_flat = mn[:].rearrange("p g x -> p (g x)")
                nc.vector.tensor_tensor_scan(
                    out=Pb[:, 1:2049], data0=mn_flat, data1=mn_flat,
                    initial=0.0, op0=ALU.add, op1=ALU.bypass)
                sh = mn  # sh overwrites mn's storage
                pv = Pb[:, 1:2049].rearrange("p (g x) -> p g x", g=G)
                nc.vector.tensor_tensor(
                    out=sh[:, :, 8:505], in0=pv[:, :, 15:512],
                    in1=pv[:, :, 0:497], op=ALU.subtract)
                for g in range(G):
                    base = g * W
                    nc.vector.tensor_tensor(
                        out=sh[:, g, 0:8], in0=Pb[:, base + 8:base + 16],
                        in1=Pb[:, base:base + 1].to_broadcast([P, 8]),
                        op=ALU.subtract)
                    nc.vector.tensor_tensor(
                        out=sh[:, g, 505:512],
                        in0=Pb[:, base + 512:base + 513].to_broadcast([P, 7]),
                        in1=Pb[:, base + 498:base + 505], op=ALU.subtract)

                dc = dc_pool.tile([P, G, W], F32, tag="dc")
                for gp in range(G):
                    ps = vband.tile([P, W], F32, tag="vps")
                    mms = [(bdiag, gp)]
                    if gp > 0:
                        mms.append((bup, gp - 1))
                    if gp < G - 1:
                        mms.append((bdn, gp + 1))
                    for j, (band, gsrc) in enumerate(mms):
                        nc.tensor.matmul(ps[:], lhsT=band, rhs=sh[:, gsrc, :],
                                         start=(j == 0), stop=(j == len(mms) - 1))
                    nc.scalar.activation(dc[:, gp, :], ps[:], ACTF.Copy,
                                         scale=1.0 / 225.0)
                return imgt, dc

            def grid_pass(i, dc_flat, thr_vec, out_col):
                """one stripe-grid counting pass + locate; writes jcnt into
                out_col [P,1] (broadcast). thr_vec: [P,1] thresholds."""
                cp = small.tile([P, 1], F32, tag="cp")
                scr = scnt.tile([P, 2 * W], F32, tag="scr")
                if i % 2 == 0:
                    nc.vector.tensor_scalar(
                        out=scr[:, :W * 2], in0=dc_flat[:, 0:2 * W],
                        scalar1=thr_vec, scalar2=None,
                        op0=ALU.is_ge, op1=ALU.add, accum_out=cp[:])
                    nc.vector.tensor_scalar(
                        out=scr[:, :W * 2], in0=dc_flat[:, 2 * W:4 * W],
                        scalar1=thr_vec, scalar2=None,
                        op0=ALU.is_ge, op1=ALU.add, accum_out=cp[:])
                    # NOTE: second accum overwrites; handled by caller variant
                return cp

            def seed_image(i, dc, lo4, wd4):
                """two-level stripe-grid seed for image i -> col of lo4/wd4
                (per-pair [P,2] state tiles, column i%2)."""
                dc_flat = dc[:].rearrange("p g x -> p (g x)")
                use_dve = (i % 2 == 0)
                # level 1
                cp = small.tile([P, 1], F32, tag="cp")
                scr = scnt.tile([P, G * W], F32, tag="scr")
                if use_dve:
                    nc.vector.tensor_scalar(
                        out=scr[:], in0=dc_flat, scalar1=g1[:], scalar2=None,
                        op0=ALU.is_ge, op1=ALU.add, accum_out=cp[:])
                    thr = THR_DVE
                else:
                    nc.scalar.activation(
                        scr[:], dc_flat, ACTF.Sign, bias=ng1[:], scale=1.0,
                        accum_out=cp[:])
                    thr = THR_ACT
                mk = small.tile([P, 1], F32, tag="mk")
                nc.vector.tensor_scalar(out=mk[:], in0=cp[:], scalar1=thr,
                                        scalar2=None, op0=ALU.is_ge)
                jc = cnt_ps.tile([P, 1], F32, tag="cps0")
                nc.tensor.matmul(jc[:], lhsT=ones, rhs=mk[:], start=True,
                                 stop=True)
                lo1 = small.tile([P, 1], F32, tag="lo1")
                nc.vector.tensor_scalar(out=lo1[:], in0=jc[:], scalar1=L1_OFF,
                                        scalar2=D1, op0=ALU.add, op1=ALU.mult)
                # level 2
                t2 = small.tile([P, 1], F32, tag="t2")
                nc.vector.scalar_tensor_tensor(
                    out=t2[:], in0=gridf[:], scalar=D2, in1=lo1[:],
                    op0=ALU.mult, op1=ALU.add)
                cp2 = small.tile([P, 1], F32, tag="cp")
                scr2 = scnt.tile([P, G * W], F32, tag="scr")
                if use_dve:
                    nc.vector.tensor_scalar(
                        out=scr2[:], in0=dc_flat, scalar1=t2[:], scalar2=None,
                        op0=ALU.is_ge, op1=ALU.add, accum_out=cp2[:])
                else:
                    nt2 = small.tile([P, 1], F32, tag="nt2")
                    nc.vector.tensor_scalar(out=nt2[:], in0=t2[:], scalar1=-1.0,
                                            scalar2=None, op0=ALU.mult)
                    nc.scalar.activation(
                        scr2[:], dc_flat, ACTF.Sign, bias=nt2[:], scale=1.0,
                        accum_out=cp2[:])
                mk2 = small.tile([P, 1], F32, tag="mk")
                nc.vector.tensor_scalar(out=mk2[:], in0=cp2[:], scalar1=thr,
                                        scalar2=None, op0=ALU.is_ge)
                jc2 = cnt_ps.tile([P, 1], F32, tag="cps0")
                nc.tensor.matmul(jc2[:], lhsT=ones, rhs=mk2[:], start=True,
                                 stop=True)
                q = small.tile([P, 1], F32, tag="q")
                nc.vector.tensor_scalar(out=q[:], in0=jc2[:], scalar1=L2_OFF,
                                        scalar2=D2, op0=ALU.add, op1=ALU.mult)
                j = i % 2
                nc.vector.tensor_tensor(out=lo4[:, j:j + 1], in0=q[:],
                                        in1=lo1[:], op=ALU.add)
                nc.vector.memset(wd4[:, j:j + 1], W2)

            def full_round(pair, dcs, lo2, wd2):
                """one bisection round for a pair; half-split DVE/ACT counts."""
                tau2 = small.tile([P, 2], F32, tag=f"tau{pair}")
                nc.vector.scalar_tensor_tensor(
                    out=tau2[:], in0=wd2[:], scalar=0.5, in1=lo2[:],
                    op0=ALU.mult, op1=ALU.add)
                ntau2 = small.tile([P, 2], F32, tag=f"ntau{pair}")
                nc.vector.tensor_scalar(
                    out=ntau2[:], in0=tau2[:], scalar1=-1.0,
                    scalar2=None, op0=ALU.mult)
                part4 = small.tile([P, 4], F32, tag=f"part{pair}")
                for j in range(2):
                    dflat = dcs[j][:].rearrange("p g x -> p (g x)")
                    scr = scnt2.tile([P, NACT_CNT], F32, tag="scr2")
                    nc.vector.tensor_scalar(
                        out=scr[:, :NDVE_CNT], in0=dflat[:, 0:NDVE_CNT],
                        scalar1=tau2[:, j:j + 1], scalar2=None,
                        op0=ALU.is_ge, op1=ALU.add,
                        accum_out=part4[:, 2 * j:2 * j + 1])
                    scr2 = scnt2.tile([P, NACT_CNT], F32, tag="scr2")
                    nc.scalar.activation(
                        scr2[:], dflat[:, NDVE_CNT:4 * W], ACTF.Sign,
                        bias=ntau2[:, j:j + 1], scale=1.0,
                        accum_out=part4[:, 2 * j + 1:2 * j + 2])
                cps = cnt_ps.tile([P, 4], F32, tag=f"cps{pair}")
                nc.tensor.matmul(cps[:], lhsT=ones, rhs=part4[:],
                                 start=True, stop=True)
                csb = small.tile([P, 4], F32, tag=f"csb{pair}")
                nc.scalar.activation(csb[:], cps[:], ACTF.Copy)
                cv = csb[:].rearrange("p (i s) -> p i s", s=2)
                u2 = small.tile([P, 2], F32, tag=f"u{pair}")
                nc.vector.scalar_tensor_tensor(
                    out=u2[:], in0=cv[:, :, 1], scalar=0.5, in1=cv[:, :, 0],
                    op0=ALU.mult, op1=ALU.add)
                a2 = small.tile([P, 2], U32, tag=f"cmp{pair}")
                nc.vector.tensor_tensor(out=a2[:], in0=u2[:], in1=kvec_full[:],
                                        op=ALU.is_ge)
                nc.vector.copy_predicated(lo2[:], a2[:], tau2[:])
                nc.vector.tensor_scalar(out=wd2[:], in0=wd2[:], scalar1=0.5,
                                        scalar2=None, op0=ALU.mult)

            def finals(i, imgt, dc, lo):
                dc_flat = dc[:].rearrange("p g x -> p (g x)")
                tm = work.tile([P, G * W], F32, tag="wk2")
                nc.vector.tensor_scalar(
                    out=tm[:], in0=dc_flat, scalar1=negw4[:, i:i + 1],
                    scalar2=1.0, op0=ALU.mult, op1=ALU.add)
                nc.vector.tensor_scalar(
                    out=tm[:], in0=tm[:], scalar1=0.001, scalar2=0.101,
                    op0=ALU.add, op1=ALU.max)
                rr = work.tile([P, G * W], F32, tag="wk3")
                nc.vector.reciprocal_approx_fast(out=rr[:], in_=tm[:])

                part4 = small.tile([P, 4], F32, tag="part4")
                nlo = small.tile([P, 1], F32, tag="nlo")
                nc.vector.tensor_scalar(out=nlo[:], in0=lo, scalar1=-1.0,
                                        scalar2=None, op0=ALU.mult)
                scrM = scnt.tile([P, G * W], F32, tag="scr")
                nc.scalar.activation(
                    scrM[:], dc_flat, ACTF.Sign, bias=nlo[:], scale=1.0,
                    accum_out=part4[:, 0:1])
                nc.vector.tensor_scalar(
                    out=part4[:, 0:1], in0=part4[:, 0:1], scalar1=2048.0,
                    scalar2=0.5, op0=ALU.add, op1=ALU.mult)
                for c in range(3):
                    scr_c = scnt.tile([P, G * W], F32, tag="scr")
                    nc.vector.scalar_tensor_tensor(
                        out=scr_c[:], in0=dc_flat, scalar=lo,
                        in1=imgt[c][:].rearrange("p g x -> p (g x)"),
                        op0=ALU.is_ge, op1=ALU.mult,
                        accum_out=part4[:, c + 1:c + 2])
                tot_ps = misc_ps.tile([P, 4], F32, tag="tot")
                nc.tensor.matmul(tot_ps[:], lhsT=ones, rhs=part4[:],
                                 start=True, stop=True)
                rcount = small.tile([P, 1], F32, tag="rcount")
                nc.vector.reciprocal(out=rcount[:], in_=tot_ps[:, 0:1])
                A3 = small.tile([P, 3], F32, tag="A3")
                nc.vector.tensor_tensor(out=A3[:], in0=tot_ps[:, 1:4],
                                        in1=rcount[:].to_broadcast([P, 3]),
                                        op=ALU.mult)

                for c in range(3):
                    img_flat = imgt[c][:].rearrange("p g x -> p (g x)")
                    d = work.tile([P, G * W], F32, tag="wk4")
                    nc.vector.scalar_tensor_tensor(
                        out=d[:], in0=img_flat, scalar=A3[:, c:c + 1], in1=rr[:],
                        op0=ALU.subtract, op1=ALU.mult)
                    nc.scalar.activation(d[:], d[:], ACTF.Relu,
                                         bias=A3[:, c:c + 1], scale=1.0)
                    nc.vector.tensor_scalar(out=img_flat, in0=d[:],
                                            scalar1=1.0, scalar2=None,
                                            op0=ALU.min)
                    nc.sync.dma_start(
                        out_d[i, c].rearrange("(g p) x -> p g x", p=P),
                        imgt[c][:])

            imgs, dcs = [], []
            for i in range(NPC):
                a, b = phase1(i)
                imgs.append(a)
                dcs.append(b)
            states = []
            for pair in range(2):
                lo2 = small.tile([P, 2], F32, tag=f"lo{pair}")
                wd2 = small.tile([P, 2], F32, tag=f"wd{pair}")
                states.append((lo2, wd2))
            for i in range(NPC):
                lo2, wd2 = states[i // 2]
                seed_image(i, dcs[i], lo2, wd2)
            for _ in range(FULL_ROUNDS):
                for pair in range(2):
                    lo2, wd2 = states[pair]
                    full_round(pair, dcs[2 * pair:2 * pair + 2], lo2, wd2)
            for i in range(NPC):
                lo2 = states[i // 2][0]
                finals(i, imgs[i], dcs[i], lo2[:, i % 2:i % 2 + 1])
    nc.compile()
    return nc


NCORES = 8
CONSTS = make_consts()
LAST_RESULT = None
_NC_CACHE = None


def _get_nc():
    global _NC_CACHE
    if _NC_CACHE is None:
        nc = bacc.Bacc("TRN2", target_bir_lowering=False, debug=False)
        _NC_CACHE = build(nc)
    return _NC_CACHE


def kernel(img: np.ndarray, w: np.ndarray) -> np.ndarray:
    global LAST_RESULT
    img = np.ascontiguousarray(np.asarray(img, dtype=np.float32))
    w = np.ascontiguousarray(np.asarray(w, dtype=np.float32))
    nc = _get_nc()
    in_maps = [
        {"img": img[i * NPC:(i + 1) * NPC], "w": w[i * NPC:(i + 1) * NPC],
         "consts": CONSTS}
        for i in range(NCORES)
    ]
    trace = bool(int(os.environ.get("DEHAZE_TRACE", "0")))
    res = run_bass_kernel_spmd(nc, in_maps, list(range(NCORES)), trace=trace)
    LAST_RESULT = res
    return np.concatenate([r["out"] for r in res.results], axis=0)

